# revision 33
# baseline (speedup 1.0000x reference)
"""Trainium2 Bass kernel for a dense transformer block.

Reference computation (B=2, T=2048, D=2048, H=16, Dk=128, FF=8192, fp32):
    h   = rmsnorm(x, g1)
    qkv = h @ w_attn.T ; q,k = rope(q,k) ; y = causal_softmax(q k^T / sqrt(Dk)) v
    x1  = x + y @ w_proj.T
    h2  = rmsnorm(x1, g2)
    out = x1 + (silu(h2 @ w_gate.T) * (h2 @ w_up.T)) @ w_down.T

Distribution: data-parallel over tokens, 512 per core (cores 0-3: batch 0,
cores 4-7: batch 1). Token tiles are "snake"-folded across the 4-core group:
core c owns global 128-token tiles {c, 7-c, 8+c, 15-c}, so every core's
causal key footprint is identical (tiles 0..3 attend 4 key tiles, 4..7
attend 8, 8..11 attend 12, 12..15 attend 16 -> 62.5% of the dense score/AV
work, perfectly balanced). Causal masking within the padded footprint is
data-driven (per-core 0/1 mask tiles multiply the exp'd scores), which keeps
the SPMD program identical on all cores.

K and V are computed locally and each moved in its own AllGather per 4-core
group; the K gather launches right after the K pass so it overlaps the V and
Q passes, and the V gather overlaps the Q pass + the first attention heads'
score work. Post-gather K/V staging DMAs are split across the sync and
gpsimd queues and prefetched several heads deep.

All weight matrices stream through one shared SBUF pool with size-class
tags reused across phases, so the DMA queue naturally prefetches the next
phase's weights while the current phase computes and no pool boundaries
stall the pipeline. Matmuls run in bf16 with fp32 PSUM accumulation.
Residuals and normalization in fp32. RoPE is applied in the transposed
[dk, t] layout via a host-side permutation of the head dimension + DVE
stream_shuffle. The softmax exp is batched two key-tiles per activation
instruction (2-bank PSUM score tiles); the denominator broadcast runs on
GpSimd (partition_broadcast) instead of a PE matmul. x1 (attention residual)
lives in SBUF end-to-end: its sum-of-squares for the second rmsnorm is
accumulated during the proj phase, and the down-projection residual add
reads it directly.
"""

import os
import sys
import threading
import time

import numpy as np

for _p in ("/opt/trn_rl_repo", os.path.expanduser("~/.axon_site/_ro/trn_rl_repo")):
    if _p not in sys.path and os.path.isdir(_p):
        sys.path.append(_p)

import ml_dtypes  # noqa: E402

import concourse.bass as bass  # noqa: E402
import concourse.mybir as mybir  # noqa: E402
import concourse.tile as tile  # noqa: E402
from concourse import bacc  # noqa: E402
from concourse.bass_utils import run_bass_kernel_spmd  # noqa: E402
from concourse.masks import make_identity  # noqa: E402
from contextlib import ExitStack  # noqa: E402

F32 = mybir.dt.float32
BF16 = mybir.dt.bfloat16
AF = mybir.ActivationFunctionType
ALU = mybir.AluOpType

B, T, D = 2, 2048, 2048
H, DK, FF = 16, 128, 8192
EPS = 1e-6
N_CORES = 8
TLOC = T * B // N_CORES          # 512 tokens per core
CORES_PER_B = N_CORES // B       # 4
KT = D // 128                    # 16 d-tiles
NT = TLOC // 128                 # 4 t-tiles per core
NKT = T // 128                   # 16 key subtiles (full sequence)
FT_FF = FF // 128                # 64 ff tiles
NFB = D // 512                   # 4 v/proj 512-col blocks
SCALE = 1.0 / float(np.sqrt(DK))
SHUF_MASK = [(j + 16) % 32 for j in range(32)]


def snake_tiles(c):
    """Global 128-token tile indices owned by group-core c, local order."""
    return [c, 7 - c, 8 + c, 15 - c]


def _gmaps():
    """global tile g -> (owning group-core, local tile index)."""
    rmap, lmap = [0] * NKT, [0] * NKT
    for g in range(NKT):
        for r in range(CORES_PER_B):
            if g in snake_tiles(r):
                rmap[g], lmap[g] = r, snake_tiles(r).index(g)
    return rmap, lmap


RMAP, LMAP = _gmaps()
# core-major position of global tile g inside gathered K/V SBUF tiles
POS = [RMAP[g] * NT + LMAP[g] for g in range(NKT)]


def _rope_perm():
    """Within-head row permutation: pair i=(16*qd + j) real part -> partition
    32*qd + j, imag part -> partition 32*qd + 16 + j."""
    perm = np.zeros(DK, dtype=np.int64)
    for p in range(DK):
        qd, j = p // 32, p % 32
        i = 16 * qd + (j if j < 16 else j - 16)
        perm[p] = 2 * i + (0 if j < 16 else 1)
    return perm


def build_program(sim=False, repeat=1):
    nc = bacc.Bacc("TRN2", target_bir_lowering=False, debug=False,
                   num_devices=1 if sim else N_CORES)

    x_d = nc.declare_dram_parameter("x", [TLOC, D], F32, isOutput=False)
    qkw_d = nc.declare_dram_parameter("qk_w", [2 * H, 128, D], BF16, isOutput=False)
    vw_d = nc.declare_dram_parameter("v_w", [KT, 128, D], BF16, isOutput=False)
    pw_d = nc.declare_dram_parameter("proj_w", [H, 128, D], BF16, isOutput=False)
    gw_d = nc.declare_dram_parameter("gate_w", [FT_FF, 128, D], BF16, isOutput=False)
    uw_d = nc.declare_dram_parameter("up_w", [FT_FF, 128, D], BF16, isOutput=False)
    dw_d = nc.declare_dram_parameter("down_w", [FT_FF, 128, D], BF16, isOutput=False)
    cs1_d = nc.declare_dram_parameter("cs1", [128, TLOC], F32, isOutput=False)
    cs2_d = nc.declare_dram_parameter("cs2", [128, TLOC], F32, isOutput=False)
    tri_d = nc.declare_dram_parameter("tri", [128, NKT * 128], BF16,
                                      isOutput=False)
    out_d = nc.declare_dram_parameter("out", [TLOC, D], F32, isOutput=True)

    with ExitStack() as ctx:
        tc = ctx.enter_context(tile.TileContext(nc))
        for _rep in range(repeat):
            _emit_block(nc, tc, sim, x_d, qkw_d, vw_d, pw_d, gw_d, uw_d, dw_d,
                        cs1_d, cs2_d, tri_d, out_d)

    nc.compile()
    return nc


def _emit_block(nc, tc, sim, x_d, qkw_d, vw_d, pw_d, gw_d, uw_d, dw_d,
                cs1_d, cs2_d, tri_d, out_d):
    with ExitStack() as ctx:
        const = ctx.enter_context(tc.tile_pool(name="const", bufs=1))
        ident = const.tile([128, 128], BF16)
        make_identity(nc, ident)
        ones_col = const.tile([128, 1], BF16)
        nc.vector.memset(ones_col, 1.0)
        cs1_sb = const.tile([128, TLOC], F32)
        nc.sync.dma_start(out=cs1_sb[:], in_=cs1_d[:, :])
        cs2_sb = const.tile([128, TLOC], F32)
        nc.sync.dma_start(out=cs2_sb[:], in_=cs2_d[:, :])
        trib_sb = const.tile([128, NKT, 128], BF16)
        nc.gpsimd.dma_start(out=trib_sb[:], in_=tri_d.rearrange(
            "p (n q) -> p n q", n=NKT))

        # one streaming pool for ALL weight tiles, with size-class tags
        # reused across phases: the DMA queues prefetch the next phase's
        # weights during the current phase's compute and there is no pool
        # boundary anywhere in the weight stream.
        wflow = ctx.enter_context(tc.tile_pool(name="wflow", bufs=6))

        # DRAM scratch: K/V allgather buffers (K gathered first, overlapping
        # the V+Q passes; V gather overlaps Q + early attention scores).
        dram = ctx.enter_context(tc.tile_pool(name="dram", bufs=1, space="DRAM"))
        # merged K+V allgather payload: entries 0..H-1 = K heads,
        # H + fb*NT + l = V block (fb, local tile l). One collective per
        # block: each collective launch carries substantial fixed runtime
        # cost on this hardware (measured ~40-90us), outweighing the
        # overlap a split K/V gather would buy.
        kv_local = dram.tile([2 * H, 128, TLOC], BF16)
        kv_full = dram.tile([CORES_PER_B, 2 * H, 128, TLOC], BF16)

        def rmsnorm_stats(pool, ssqs):
            """ssqs: list of [128,1] f32 sum-of-square tiles (one per it,
            possibly pre-accumulated) -> list of rstd tiles."""
            rstds = []
            for it in range(NT):
                mean = pool.tile([128, 1], F32, name="mean")
                nc.vector.tensor_scalar(mean[:], ssqs[it][:], 1.0 / D, EPS,
                                        ALU.mult, ALU.add)
                rec = pool.tile([128, 1], F32, name="rec")
                nc.vector.reciprocal(rec[:], mean[:])
                rstd = pool.tile([128, 1], F32, name=f"rstd{it}",
                                 tag=f"rstd{it}", bufs=1)
                nc.scalar.activation(rstd[:], rec[:], AF.Sqrt)
                rstds.append(rstd)
            return rstds

        def norm_rows_transpose(loader, rstds, dst_sb, pool, psum_pool):
            """dst_sb[:, k, it*128:...] = transpose of loader(it)*rstd[it]."""
            hrows = []
            for it in range(NT):
                hrow = pool.tile([128, D], BF16, name=f"hrow{it}",
                                 tag=f"hrow{it}", bufs=1)
                nc.vector.tensor_scalar(hrow[:], loader(it), rstds[it][:],
                                        None, ALU.mult)
                hrows.append(hrow)
            # k-outer so dst_sb[:, k, :] completes early for the consumers
            for k in range(KT):
                for it in range(NT):
                    tp = psum_pool.tile([128, 128], BF16, name="tp")
                    nc.tensor.transpose(tp[:], hrows[it][:, k * 128:(k + 1) * 128],
                                        ident[:])
                    nc.vector.tensor_copy(dst_sb[:, k, it * 128:(it + 1) * 128],
                                          tp[:])

        def rope_evict(ps, dst, pool):
            """ps: [128, TLOC] psum q/k head tile (permuted lanes) -> rotated"""
            sh = pool.tile([128, TLOC], F32, name="rp_sh")
            nc.vector.stream_shuffle(sh[:], ps[:], mask=SHUF_MASK)
            t1 = pool.tile([128, TLOC], F32, name="rp_t1")
            nc.vector.tensor_tensor(t1[:], ps[:], cs1_sb[:], ALU.mult)
            t2 = pool.tile([128, TLOC], F32, name="rp_t2")
            nc.vector.tensor_tensor(t2[:], sh[:], cs2_sb[:], ALU.mult)
            nc.vector.tensor_tensor(dst[:], t1[:], t2[:], ALU.add)

        # persistent pools, strict LIFO (longest-lived first)
        x1_cm = tc.tile_pool(name="x1_pool", bufs=1)
        x1_pool = x1_cm.__enter__()
        x1_sb = x1_pool.tile([128, NT, D], F32)
        ssq_cm = tc.tile_pool(name="ssqp_pool", bufs=1)
        ssqp_pool = ssq_cm.__enter__()
        qrot_cm = tc.tile_pool(name="qrot_pool", bufs=1)
        qrot_pool = qrot_cm.__enter__()
        qrot_sb = qrot_pool.tile([128, H, TLOC], BF16)
        hT_cm = tc.tile_pool(name="hT_pool", bufs=1)
        hT_pool = hT_cm.__enter__()
        hT_sb = hT_pool.tile([128, KT, TLOC], BF16)

        # ---------------- phase 1: norm1 + h^T ----------------
        x_cm = tc.tile_pool(name="xpool", bufs=1)
        xpool = x_cm.__enter__()
        x_sb = xpool.tile([128, NT, D], F32)
        for it in range(NT):
            eng = nc.sync if it % 2 == 0 else nc.gpsimd
            eng.dma_start(out=x_sb[:, it, :],
                          in_=x_d[it * 128:(it + 1) * 128, :])
        # prefetch the first K-head weight tiles behind the x loads
        qk_wts = {}
        for h in range(2):
            wt = wflow.tile([128, KT, 128], BF16, name="qk_wt", tag="w4k")
            nc.sync.dma_start(out=wt[:], in_=qkw_d[H + h].rearrange(
                "p (k c) -> p k c", k=KT))
            qk_wts[h] = wt
        with ExitStack() as ph:
            pool = ph.enter_context(tc.tile_pool(name="n1_pool", bufs=2))
            psum_pool = ph.enter_context(
                tc.tile_pool(name="n1_psum", bufs=4, space="PSUM"))
            ssqs = []
            for it in range(NT):
                sq_scr = pool.tile([128, D], BF16, name="sq_scr")
                ssq = pool.tile([128, 1], F32, name=f"ssq{it}", tag=f"ssq{it}",
                                bufs=1)
                nc.scalar.activation(sq_scr[:], x_sb[:, it, :], AF.Square,
                                     accum_out=ssq[:])
                ssqs.append(ssq)
            rstds = rmsnorm_stats(pool, ssqs)
            norm_rows_transpose(lambda it: x_sb[:, it, :], rstds, hT_sb,
                                pool, psum_pool)
        x_cm.__exit__(None, None, None)

        # ---------------- phase 2a: K heads + K allgather ----------------
        with ExitStack() as ph:
            spool = ph.enter_context(tc.tile_pool(name="k_s", bufs=3))
            pspool = ph.enter_context(
                tc.tile_pool(name="k_ps", bufs=3, space="PSUM"))
            for h in range(H):
                if h in qk_wts:
                    wt = qk_wts.pop(h)
                else:
                    wt = wflow.tile([128, KT, 128], BF16, name="qk_wt",
                                    tag="w4k")
                    nc.sync.dma_start(out=wt[:], in_=qkw_d[H + h].rearrange(
                        "p (k c) -> p k c", k=KT))
                ps = pspool.tile([128, TLOC], F32, name="qk_ps")
                for k in range(KT):
                    nc.tensor.matmul(ps[:], wt[:, k, :], hT_sb[:, k, :],
                                     start=(k == 0), stop=(k == KT - 1))
                krot = spool.tile([128, TLOC], BF16, name="krot")
                rope_evict(ps, krot[:], spool)
                # store via the Activation HWDGE queue: the gpsimd queue is
                # reserved for the collective, which occupies it end-to-end
                nc.scalar.dma_start(out=kv_local[h], in_=krot[:])

        kt_cm = tc.tile_pool(name="ktpool", bufs=5)
        ktpool = kt_cm.__enter__()

        # Engine sequencers have only a ~4-deep wait queue for instructions
        # with unsatisfied dependencies, so collective/slot-blocked staging
        # DMAs are spread so no engine ever holds more than 2 of them —
        # otherwise they head-of-line block unrelated work on that engine.
        def stage_kT(h, engs):
            kT_sb = ktpool.tile([128, T], BF16, name="kT_sb")
            for r in range(CORES_PER_B):
                engs[r].dma_start(out=kT_sb[:, r * TLOC:(r + 1) * TLOC],
                                  in_=kv_full[r, h])
            return kT_sb

        kT_tiles = {}

        # V columns for attention rotate per 512-col block (2 resident:
        # heads 4fb..4fb+3 consume block fb while fb+1 streams in)
        vall_cm = tc.tile_pool(name="vall_pool", bufs=2)
        vap = vall_cm.__enter__()
        v_fbs = []

        # ---------------- phase 2b: V + merged KV allgather --------------
        with ExitStack() as ph:
            spool = ph.enter_context(tc.tile_pool(name="v_s", bufs=3))
            vpspool = ph.enter_context(
                tc.tile_pool(name="v_psp", bufs=1, space="PSUM"))
            for fb in range(NFB):
                vps = [vpspool.tile([128, 512], F32, name=f"v_ps{it}",
                                    tag=f"v_ps{it}") for it in range(NT)]
                for k in range(KT):
                    vwt = wflow.tile([128, 512], BF16, name="vwt", tag="w1k")
                    nc.sync.dma_start(out=vwt[:],
                                      in_=vw_d[k][:, fb * 512:(fb + 1) * 512])
                    for it in range(NT):
                        nc.tensor.matmul(vps[it][:],
                                         hT_sb[:, k, it * 128:(it + 1) * 128],
                                         vwt[:], start=(k == 0), stop=(k == KT - 1))
                for it in range(NT):
                    vsb = spool.tile([128, 512], BF16, name="vsb")
                    nc.scalar.copy(vsb[:], vps[it][:])
                    nc.scalar.dma_start(
                        out=kv_local[H + fb * NT + it], in_=vsb[:])

        if sim:
            for r in range(CORES_PER_B):
                nc.gpsimd.dma_start(out=kv_full[r], in_=kv_local[:])
        else:
            nc.gpsimd.collective_compute(
                "AllGather", ALU.bypass,
                replica_groups=[[0, 1, 2, 3], [4, 5, 6, 7]],
                ins=[kv_local.opt()], outs=[kv_full.opt()],
            )

        # stage the first heads' K columns; these DMAs fire the moment the
        # gather completes, overlapping the Q pass tail
        kT_tiles[0] = stage_kT(0, [nc.sync, nc.sync, nc.scalar, nc.scalar])
        kT_tiles[1] = stage_kT(1, [nc.gpsimd, nc.gpsimd, nc.sync, nc.sync])

        def stage_vfb(engs):
            v_fb = vap.tile([128, NKT, 512], BF16, name="v_fb")
            fb = len(v_fbs)
            for r in range(CORES_PER_B):
                engs[r].dma_start(
                    out=v_fb[:, r * NT:(r + 1) * NT, :],
                    in_=kv_full[r, H + fb * NT:H + fb * NT + NT].rearrange(
                        "l p c -> p l c"))
            v_fbs.append(v_fb)

        stage_vfb([nc.scalar, nc.scalar, nc.scalar, nc.scalar])
        stage_vfb([nc.sync, nc.sync, nc.scalar, nc.scalar])

        # ---------------- phase 2c: Q heads + rope (gathers overlap) -----
        with ExitStack() as ph:
            spool = ph.enter_context(tc.tile_pool(name="q_s", bufs=3))
            pspool = ph.enter_context(
                tc.tile_pool(name="q_ps", bufs=3, space="PSUM"))
            for h in range(H):
                wt = wflow.tile([128, KT, 128], BF16, name="q_wt", tag="w4k")
                nc.sync.dma_start(out=wt[:], in_=qkw_d[h].rearrange(
                    "p (k c) -> p k c", k=KT))
                ps = pspool.tile([128, TLOC], F32, name="qk_ps")
                for k in range(KT):
                    nc.tensor.matmul(ps[:], wt[:, k, :], hT_sb[:, k, :],
                                     start=(k == 0), stop=(k == KT - 1))
                rope_evict(ps, qrot_sb[:, h, :], spool)

        y_cm = tc.tile_pool(name="y_pool", bufs=1)
        y_pool = y_cm.__enter__()
        y_sb = y_pool.tile([128, H, TLOC], BF16)

        # ---------------- phase 3: attention (snake-folded causal) -------
        # key-tile pair j covers kts (2j, 2j+1); both share the same local
        # query window [c0:512) with c0=(2j//4)*128, so their scores live in
        # one 2-bank PSUM tile and one Exp activation handles both. The
        # first 128 query cols of each kt get the data-driven causal mask.
        with ExitStack() as ph:
            apool = ph.enter_context(tc.tile_pool(name="att_pool", bufs=2))
            epool = ph.enter_context(tc.tile_pool(name="exp_pool", bufs=6))
            aps = ph.enter_context(tc.tile_pool(name="att_ps", bufs=2, space="PSUM"))
            sps_pool = ph.enter_context(
                tc.tile_pool(name="sps_pool", bufs=2, space="PSUM"))

            def emit_av(em, j, h, yps, sums):
                c0 = (2 * j // 4) * 128
                w = TLOC - c0
                for i in range(2):
                    kt = 2 * j + i
                    kp = POS[kt]
                    nc.tensor.matmul(yps[:, c0:TLOC],
                                     v_fbs[h // 4][:, kp,
                                                   (h % 4) * 128:
                                                   (h % 4 + 1) * 128],
                                     em[:, i, :w], start=(kt == 0),
                                     stop=(kt == NKT - 1),
                                     skip_group_check=True)
                    nc.tensor.matmul(sums[:, c0:TLOC], ones_col[:],
                                     em[:, i, :w], start=(kt == 0),
                                     stop=(kt == NKT - 1),
                                     skip_group_check=True)
                if j == NKT // 2 - 1:
                    rec = apool.tile([1, TLOC], F32, name="rec_att")
                    nc.vector.reciprocal(rec[:], sums[:])
                    bco = apool.tile([128, TLOC], F32, name="bco")
                    nc.gpsimd.partition_broadcast(bco[:], rec[:], channels=128)
                    nc.vector.tensor_tensor(y_sb[:, h, :], yps[:], bco[:],
                                            ALU.mult)

            # software-pipelined one pair deep ACROSS heads: the AV/sums
            # matmuls of the previous pair issue after the current pair's
            # score matmuls, so the PE never waits on the Exp it just
            # scheduled — including across head boundaries.
            pending = None
            for h in range(H):
                # late v-blocks are staged mid-loop (their slots free only
                # after heads 3/7), split 2+2 across engines so the blocked
                # DMAs never fill an engine's 4-deep wait window
                if h in (1, 5):
                    stage_vfb([nc.scalar, nc.scalar, nc.gpsimd, nc.gpsimd])
                if h in kT_tiles:
                    kT_sb = kT_tiles.pop(h)
                else:
                    kT_sb = stage_kT(h, [nc.sync, nc.sync,
                                         nc.gpsimd, nc.gpsimd])
                yps = aps.tile([128, TLOC], F32, name="y_ps", tag="y_ps")
                sums = aps.tile([1, TLOC], F32, name="sums_ps", tag="sums_ps")
                for j in range(NKT // 2):
                    c0 = (2 * j // 4) * 128
                    w = TLOC - c0
                    sps = sps_pool.tile([128, 2, 512], F32, name="s_ps",
                                        tag="s_ps")
                    for i in range(2):
                        kp = POS[2 * j + i]
                        nc.tensor.matmul(sps[:, i, :w],
                                         kT_sb[:, kp * 128:(kp + 1) * 128],
                                         qrot_sb[:, h, c0:TLOC], start=True,
                                         stop=True)
                    em = epool.tile([128, 2, 512], BF16, name="em")
                    nc.scalar.activation(em[:, 0:2, :w], sps[:, 0:2, :w],
                                         AF.Exp, scale=SCALE)
                    for i in range(2):
                        kt = 2 * j + i
                        nc.vector.tensor_tensor(em[:, i, 0:128], em[:, i, 0:128],
                                                trib_sb[:, kt, :], ALU.mult)
                    if pending is not None:
                        emit_av(*pending)
                    pending = (em, j, h, yps, sums)
            emit_av(*pending)

        # ------- phase 4: proj + residual -> x1 (SBUF-resident) ----------
        # the second rmsnorm's sum-of-squares accumulates here per
        # (it, fb) partial so norm2 has almost nothing left on the
        # critical path.
        ssq_parts = [[None] * NFB for _ in range(NT)]
        with ExitStack() as ph:
            spool = ph.enter_context(tc.tile_pool(name="pj_s", bufs=4))
            pps = ph.enter_context(tc.tile_pool(name="pj_ps", bufs=2, space="PSUM"))
            for fb in range(NFB):
                pps_t = [pps.tile([128, 512], F32, name=f"p_ps{it}",
                                  tag=f"p_ps{it}") for it in range(NT)]
                for hd in range(H):
                    pwt = wflow.tile([128, 512], BF16, name="pwt", tag="w1k")
                    nc.sync.dma_start(out=pwt[:],
                                      in_=pw_d[hd][:, fb * 512:(fb + 1) * 512])
                    for it in range(NT):
                        nc.tensor.matmul(pps_t[it][:],
                                         y_sb[:, hd, it * 128:(it + 1) * 128],
                                         pwt[:], start=(hd == 0),
                                         stop=(hd == H - 1))
                for it in range(NT):
                    xr = spool.tile([128, 512], F32, name="xr_p")
                    eng = nc.gpsimd if it % 2 == 0 else nc.sync
                    eng.dma_start(
                        out=xr[:],
                        in_=x_d[it * 128:(it + 1) * 128,
                                fb * 512:(fb + 1) * 512])
                    x1t = x1_sb[:, it, fb * 512:(fb + 1) * 512]
                    nc.vector.tensor_tensor(x1t, pps_t[it][:], xr[:], ALU.add)
                    sq_scr = spool.tile([128, 512], BF16, name="sq_scr2")
                    ssqp = ssqp_pool.tile([128, 1], F32,
                                          name=f"sp{it}_{fb}",
                                          tag=f"sp{it}_{fb}")
                    nc.scalar.activation(sq_scr[:], x1t, AF.Square,
                                         accum_out=ssqp[:])
                    ssq_parts[it][fb] = ssqp

        y_cm.__exit__(None, None, None)
        vall_cm.__exit__(None, None, None)
        kt_cm.__exit__(None, None, None)
        hT_cm.__exit__(None, None, None)
        qrot_cm.__exit__(None, None, None)

        # ---------------- phase 5: norm2 + h2^T (stats precomputed) ------
        h2T_cm = tc.tile_pool(name="h2T_pool", bufs=1)
        h2T_pool = h2T_cm.__enter__()
        h2T_sb = h2T_pool.tile([128, KT, TLOC], BF16)
        with ExitStack() as ph:
            pool = ph.enter_context(tc.tile_pool(name="n2_pool", bufs=2))
            psum_pool = ph.enter_context(
                tc.tile_pool(name="n2_psum", bufs=4, space="PSUM"))
            ssqs = []
            for it in range(NT):
                acc01 = pool.tile([128, 1], F32, name="acc01")
                nc.vector.tensor_tensor(acc01[:], ssq_parts[it][0][:],
                                        ssq_parts[it][1][:], ALU.add)
                acc23 = pool.tile([128, 1], F32, name="acc23")
                nc.vector.tensor_tensor(acc23[:], ssq_parts[it][2][:],
                                        ssq_parts[it][3][:], ALU.add)
                ssq = pool.tile([128, 1], F32, name=f"ssqb{it}",
                                tag=f"ssqb{it}", bufs=1)
                nc.vector.tensor_tensor(ssq[:], acc01[:], acc23[:], ALU.add)
                ssqs.append(ssq)
            rstds = rmsnorm_stats(pool, ssqs)
            norm_rows_transpose(lambda it: x1_sb[:, it, :], rstds, h2T_sb,
                                pool, psum_pool)

        # ---------------- phase 6: gate/up ----------------
        gu_cm = tc.tile_pool(name="gu_pool", bufs=1)
        gu_pool = gu_cm.__enter__()
        gu_sb = gu_pool.tile([128, FT_FF, TLOC], BF16)
        with ExitStack() as ph:
            spool = ph.enter_context(tc.tile_pool(name="mlp_s", bufs=3))
            mps = ph.enter_context(tc.tile_pool(name="mlp_ps", bufs=4, space="PSUM"))
            for f in range(FT_FF):
                gwt = wflow.tile([128, KT, 128], BF16, name="gwt", tag="w4k")
                nc.sync.dma_start(out=gwt[:], in_=gw_d[f].rearrange(
                    "p (k c) -> p k c", k=KT))
                gps = mps.tile([128, TLOC], F32, name="g_ps", tag="g_ps")
                for k in range(KT):
                    nc.tensor.matmul(gps[:], gwt[:, k, :], h2T_sb[:, k, :],
                                     start=(k == 0), stop=(k == KT - 1))
                gsil = spool.tile([128, TLOC], BF16, name="gsil")
                nc.scalar.activation(gsil[:], gps[:], AF.Silu)
                uwt = wflow.tile([128, KT, 128], BF16, name="uwt", tag="w4k")
                nc.sync.dma_start(out=uwt[:], in_=uw_d[f].rearrange(
                    "p (k c) -> p k c", k=KT))
                ups = mps.tile([128, TLOC], F32, name="u_ps", tag="u_ps")
                for k in range(KT):
                    nc.tensor.matmul(ups[:], uwt[:, k, :], h2T_sb[:, k, :],
                                     start=(k == 0), stop=(k == KT - 1))
                nc.vector.tensor_tensor(gu_sb[:, f, :], ups[:], gsil[:],
                                        ALU.mult)

        # ---------------- phase 7: down + residual -> out ----------------
        with ExitStack() as ph:
            spool = ph.enter_context(tc.tile_pool(name="dn_s", bufs=8))
            dps = ph.enter_context(tc.tile_pool(name="dn_ps", bufs=1, space="PSUM"))
            for fbp in range(2):
                dps_t = [[dps.tile([128, 512], F32, name=f"d_ps{it}_{fbi}",
                                   tag=f"d_ps{it}_{fbi}") for fbi in range(2)]
                         for it in range(NT)]
                for k in range(FT_FF):
                    dwt = wflow.tile([128, 1024], BF16, name="dwt", tag="w2k")
                    nc.sync.dma_start(
                        out=dwt[:],
                        in_=dw_d[k][:, fbp * 1024:(fbp + 1) * 1024])
                    for it in range(NT):
                        for fbi in range(2):
                            nc.tensor.matmul(
                                dps_t[it][fbi][:],
                                gu_sb[:, k, it * 128:(it + 1) * 128],
                                dwt[:, fbi * 512:(fbi + 1) * 512],
                                start=(k == 0), stop=(k == FT_FF - 1))
                for it in range(NT):
                    for fbi in range(2):
                        fb = fbp * 2 + fbi
                        osb = spool.tile([128, 512], F32, name="osb_d")
                        nc.vector.tensor_tensor(
                            osb[:], dps_t[it][fbi][:],
                            x1_sb[:, it, fb * 512:(fb + 1) * 512], ALU.add)
                        eng = nc.sync if fbi % 2 == 0 else nc.gpsimd
                        eng.dma_start(
                            out=out_d[it * 128:(it + 1) * 128,
                                      fb * 512:(fb + 1) * 512],
                            in_=osb[:])

        gu_cm.__exit__(None, None, None)
        h2T_cm.__exit__(None, None, None)
        ssq_cm.__exit__(None, None, None)
        x1_cm.__exit__(None, None, None)


def core_token_idx(c):
    """Global token indices (within the batch row) owned by group-core c."""
    return np.concatenate([np.arange(g * 128, (g + 1) * 128)
                           for g in snake_tiles(c)])


def prepare_inputs(x, f_cos, f_sin, w_attn, w_proj, w_gate, w_up, w_down, g1, g2):
    """Host-side sharding + weight re-layout. Returns list of 8 input dicts."""
    x = np.asarray(x, dtype=np.float32)
    f_cos = np.asarray(f_cos, dtype=np.float32)
    f_sin = np.asarray(f_sin, dtype=np.float32)
    w_attn = np.asarray(w_attn, dtype=np.float32)
    g1 = np.asarray(g1, dtype=np.float32)
    g2 = np.asarray(g2, dtype=np.float32)

    perm = _rope_perm()
    wq = w_attn[0:D] * g1[None, :]
    wk = w_attn[D:2 * D] * g1[None, :]
    wv = w_attn[2 * D:3 * D] * g1[None, :]
    # permute rows within each head for q and k
    wq_p = wq.reshape(H, DK, D)[:, perm, :].reshape(H * DK, D)
    wk_p = wk.reshape(H, DK, D)[:, perm, :].reshape(H * DK, D)

    def lhsT_layout(w):  # w: [F, D] -> [F/128, 128(d within k-tile), D(k*128+c)]
        f = w.shape[0]
        # out[ft, p, k*128+c] = w[ft*128+c, k*128+p]
        a = w.reshape(f // 128, 128, KT, 128)       # [ft, c, k, p]
        a = a.transpose(0, 3, 2, 1).reshape(f // 128, 128, D)  # [ft, p, (k c)]
        return np.ascontiguousarray(a).astype(ml_dtypes.bfloat16)

    def rhsT_layout(w):  # w: [F, D_in] -> [D_in/128, 128(p), F] = w.T tiled
        d_in = w.shape[1]
        a = w.T.reshape(d_in // 128, 128, w.shape[0])  # [k, p, c]
        return np.ascontiguousarray(a).astype(ml_dtypes.bfloat16)

    qk_w = np.concatenate([lhsT_layout(wq_p), lhsT_layout(wk_p)], axis=0)
    v_w = rhsT_layout(wv)
    proj_w = rhsT_layout(np.asarray(w_proj, dtype=np.float32))
    gate_w = lhsT_layout(np.asarray(w_gate, dtype=np.float32) * g2[None, :])
    up_w = lhsT_layout(np.asarray(w_up, dtype=np.float32) * g2[None, :])
    down_w = rhsT_layout(np.asarray(w_down, dtype=np.float32))

    # cs1/cs2 in permuted-lane layout: [128, T]
    pair = np.zeros(DK, dtype=np.int64)
    sign = np.zeros(DK, dtype=np.float32)
    for p in range(DK):
        qd, j = p // 32, p % 32
        pair[p] = 16 * qd + (j if j < 16 else j - 16)
        sign[p] = -1.0 if j < 16 else 1.0
    cs1_full = f_cos.T[pair, :]                       # [128, T]
    cs2_full = f_sin.T[pair, :] * sign[:, None]       # [128, T]

    in_maps = []
    for core in range(N_CORES):
        b, c = core // CORES_PER_B, core % CORES_PER_B
        tok = core_token_idx(c)
        tiles = snake_tiles(c)
        # causal mask tiles: kt covers query tile l=kt//4 (this core's
        # global tile tiles[l]); allowed iff key_pos <= query_pos
        tri = np.zeros((NKT, 128, 128), dtype=np.float32)
        kk = np.arange(128)[:, None]
        qq = np.arange(128)[None, :]
        for kt in range(NKT):
            g = tiles[kt // 4]
            tri[kt] = (kt * 128 + kk) <= (g * 128 + qq)
        tri = np.ascontiguousarray(
            tri.transpose(1, 0, 2).reshape(128, NKT * 128))
        in_maps.append({
            "x": np.ascontiguousarray(x[b, tok, :]),
            "qk_w": qk_w, "v_w": v_w, "proj_w": proj_w,
            "gate_w": gate_w, "up_w": up_w, "down_w": down_w,
            "cs1": np.ascontiguousarray(cs1_full[:, tok]),
            "cs2": np.ascontiguousarray(cs2_full[:, tok]),
            "tri": tri.astype(ml_dtypes.bfloat16),
        })
    return in_maps


def assemble_output(results):
    out = np.zeros((B, T, D), dtype=np.float32)
    for core in range(N_CORES):
        b, c = core // CORES_PER_B, core % CORES_PER_B
        out[b, core_token_idx(c), :] = results[core]["out"]
    return out


_CACHE = {}
_LOCK = threading.Lock()


def get_program():
    with _LOCK:
        if "nc" not in _CACHE:
            _CACHE["nc"] = build_program()
        return _CACHE["nc"]


def kernel(**inputs):
    nc = get_program()
    in_maps = prepare_inputs(**inputs)
    res = run_bass_kernel_spmd(nc, in_maps, list(range(N_CORES)))
    return assemble_output(res.results)


def bench(inputs, iters=10):
    """Wall-clock the sharded executable with device-resident inputs.

    Returns the mean pipelined per-call time in ns (upper bound on HW exec
    time: it includes 1/iters of the axon dispatch round-trip)."""
    import jax
    from jax.sharding import Mesh, PartitionSpec, NamedSharding
    from jax.experimental.shard_map import shard_map
    from concourse import bass2jax, mybir as mb

    nc = get_program()
    in_maps = prepare_inputs(**inputs)
    bass2jax.install_neuronx_cc_hook()

    partition_name = (nc.partition_id_tensor.name
                      if nc.partition_id_tensor else None)
    in_names, out_names, out_avals, zero_outs = [], [], [], []
    for alloc in nc.m.functions[0].allocations:
        if not isinstance(alloc, mb.MemoryLocationSet):
            continue
        name = alloc.memorylocations[0].name
        if alloc.kind == "ExternalInput":
            if name != partition_name:
                in_names.append(name)
        elif alloc.kind == "ExternalOutput":
            shape = tuple(alloc.tensor_shape)
            dtype = mb.dt.np(alloc.dtype)
            out_names.append(name)
            out_avals.append(jax.core.ShapedArray(shape, dtype))
            zero_outs.append(np.zeros(shape, dtype))
    n_params = len(in_names)
    all_in_names = list(in_names) + list(out_names)
    if partition_name is not None:
        all_in_names.append(partition_name)
    donate = tuple(range(n_params, n_params + len(out_names)))

    def _body(*args):
        operands = list(args)
        if partition_name is not None:
            operands.append(bass2jax.partition_id_tensor())
        return tuple(bass2jax._bass_exec_p.bind(
            *operands,
            out_avals=tuple(out_avals),
            in_names=tuple(all_in_names),
            out_names=tuple(out_names),
            lowering_input_output_aliases=(),
            sim_require_finite=True,
            sim_require_nnan=True,
            nc=nc,
        ))

    devices = jax.devices()[:N_CORES]
    mesh = Mesh(np.asarray(devices), ("core",))
    in_specs = (PartitionSpec("core"),) * (n_params + len(out_names))
    out_specs = (PartitionSpec("core"),) * len(out_names)
    sharded = jax.jit(
        shard_map(_body, mesh=mesh, in_specs=in_specs, out_specs=out_specs,
                  check_rep=False),
        donate_argnums=donate, keep_unused=True)

    sh = NamedSharding(mesh, PartitionSpec("core"))
    concat_in = [
        jax.device_put(
            np.concatenate([np.asarray(in_maps[c][nm]) for c in range(N_CORES)],
                           axis=0), sh)
        for nm in in_names]
    jax.block_until_ready(concat_in)

    def make_zeros():
        return [jax.device_put(
            np.zeros((N_CORES * z.shape[0], *z.shape[1:]), z.dtype), sh)
            for z in zero_outs]

    # warmup (compile)
    outs = sharded(*concat_in, *make_zeros())
    jax.block_until_ready(outs)

    zs = [make_zeros() for _ in range(iters)]
    for z in zs:
        jax.block_until_ready(z)
    # async pipelined dispatch amortizes the ~100ms axon round-trip
    t0 = time.perf_counter()
    outs = [sharded(*concat_in, *zs[i]) for i in range(iters)]
    jax.block_until_ready(outs)
    dt = (time.perf_counter() - t0) / iters
    return dt * 1e9
